# revision 1
# baseline (speedup 1.0000x reference)
"""Cost volume (tfa CorrelationCost, kernel_size=1, d=4) on 8 TRN2 cores.

out[b, k, y, x] = (1/C) * sum_c prv[b,c,y,x] * nxt_pad[b,c,y+dy,x+dx],
k = dy*9+dx, dy/dx in 0..8, nxt zero-padded by 4 on each spatial side.

Sharding: core i -> (batch b = i//2, H-half h = i%2). Each core gets the
full-C feature maps for its 64 rows (prv) and 72 padded rows (nxt).

Per-core algorithm (fp16 banded matmul), v4 — HBM-traffic-minimized:
  - pixels are tiled into 16y x 8x = 128 blocks -> lhsT [C=128, 128pix]
  - rhs = the 24y' x 16x' window of UNBANDED padded nxt ([C, 72, 264] in
    SBUF) via a 2-free-dim access pattern -> no x-band duplication
  - one matmul per tile: psum[pix, n=wy*16+wx]; pixel (q,r) (q=row 0..15,
    r=col 0..7, partition m=8q+r) needs n=(q+dy)*16+(r+dx), i.e. window
    rows wy in [q, q+9) only -> its 81 useful values live in a 9-row slab
  - evac (fp32->fp16, vector/scalar alternating per y-band) writes stage
    interleaved at wy-block granularity: stage[part, yb, wy, xb, wx], so
    each evac writes 24 near-contiguous 32B runs (fast) while the
    out-DMA slab for (yb, q) = wy-rows [q, q+9) x all xb x wx is ONE
    contiguous 9216B run per partition (full DMA line rate)
  - out traffic: 128pix * 9*16 cols * 2B = 4.72 MB vs 12.6 MB full dump.

Traffic per core: prv 4.19 MB + nxt 4.86 MB + out 4.72 MB = 13.8 MB
(vs 23.9 MB v1) -> ~38.5 us at the 358 GB/s HBM-per-core roofline.

Engine plan: first-band inputs on the sync HWDGE ring (starts ~0.6us),
the rest via gpsimd SWDGE; out slab DMAs on the two HWDGE rings (SP for
vector-evac'd bands, ACT for scalar-evac'd bands, emitted after that
band's evacs so the FIFO never stalls an evac); dummy ldweights absorb
input-DMA waits so each matmul carries only its psum-slot release wait.
"""

import numpy as np

import bass_rust
import concourse.bass as bass
import concourse.tile as tile
from concourse import bacc, mybir
from concourse.bass_utils import run_bass_kernel_spmd

# Problem geometry (hardcoded per spec)
B, C, H, W = 4, 128, 128, 256
D = 4
ND = 2 * D + 1            # 9
K = ND * ND               # 81
HH = H // 2               # 64 rows per core
HP = HH + 2 * D           # 72 padded nxt rows per core
WP = W + 2 * D            # 264 padded nxt cols
YB, XB = 16, 8            # pixel tile: 16 rows x 8 cols = 128 partitions
NY, NX = YB + 2 * D, XB + 2 * D   # 24 x 16 window
NTY, NTX = HH // YB, W // XB      # 4 y-bands x 32 x-tiles
NWIN = NY * NX            # 384
SLAB = ND * NX            # 144 useful cols per pixel-row q
N_CORES = 8

F16 = mybir.dt.float16
F32 = mybir.dt.float32


def build_nc():
    nc = bacc.Bacc("TRN2")
    prv_d = nc.declare_dram_parameter("prv_s", [C, NTY * NTX * 128], F16, isOutput=False)
    nxt_d = nc.declare_dram_parameter("nxt_s", [C, HP * WP], F16, isOutput=False)
    # Output in two waves: bands 0..2 (overlapped with band-3 compute)
    # and band 3 (the tail).
    out_a = nc.declare_dram_parameter(
        "out_a", [YB, XB * 3 * ND * NTX * NX], F16, isOutput=True
    )
    out_b = nc.declare_dram_parameter(
        "out_b", [YB, XB * ND * NTX * NX], F16, isOutput=True
    )

    with tile.TileContext(nc) as tc:
        with (
            tc.tile_pool(name="inp", bufs=1) as inp,
            tc.tile_pool(name="psum", bufs=8, space="PSUM") as pp,
            tc.tile_pool(name="stage", bufs=1) as sp,
        ):
            prv_sb = inp.tile([C, NTY * NTX * 128], F16)
            nxt_sb = inp.tile([C, HP, WP], F16)
            # wy-block-interleaved stage: [part, yb, wy, xb, wx]. Evac of
            # tile (yb, xb) writes 24 runs of 16 elems (32B, stride 512
            # elems); the (yb, q) out-slab is wy in [q, q+9) -> one
            # contiguous 9*32*16 = 4608-elem (9216B) run per partition.
            stage = sp.tile([128, NTY, NY, NTX, NX], F16)

            # Inputs: DMA fixed cost is ~0.4us per dma_start across the
            # engine set, so keep the count low: 3 nxt chunks of 24 rows
            # (band yb needs rows [16yb, 16yb+24) -> chunks align with
            # band needs) + 4 per-band prv chunks. First-band deps go on
            # the HWDGE (sync) ring; the rest via gpsimd SWDGE.
            def nxt_chunk(j, eng):  # 24-row chunks
                eng.dma_start(
                    nxt_sb[:, 24 * j : 24 * j + 24, :],
                    nxt_d[:, 24 * j * WP : (24 * j + 24) * WP],
                )

            def prv_chunk(j, eng):  # per-band chunks of 32 tiles
                lo = j * NTX * 128
                eng.dma_start(
                    prv_sb[:, lo : lo + NTX * 128], prv_d[:, lo : lo + NTX * 128]
                )

            nxt_chunk(0, nc.sync)
            prv_chunk(0, nc.sync)
            nxt_chunk(1, nc.gpsimd)
            prv_chunk(1, nc.gpsimd)
            nxt_chunk(2, nc.gpsimd)
            prv_chunk(2, nc.gpsimd)
            prv_chunk(3, nc.gpsimd)

            for yb in range(NTY):
                # Absorb input-DMA waits on cheap PE instructions so each
                # matmul below carries only its psum-release wait. Band yb
                # reads nxt rows [16yb, 16yb+24) -> touch both ends.
                nc.tensor.ldweights(prv_sb[:, yb * NTX * 128 : yb * NTX * 128 + 1])
                nc.tensor.ldweights(nxt_sb[:, 16 * yb, :1])
                nc.tensor.ldweights(nxt_sb[:, 16 * yb + 23, :1])
                for xb in range(NTX):
                    t = yb * NTX + xb
                    ps = pp.tile([128, NWIN], F32)
                    lhsT = prv_sb[:, t * 128 : (t + 1) * 128]
                    rhs = nxt_sb[:, yb * YB : yb * YB + NY, xb * XB : xb * XB + NX]
                    nc.tensor.matmul(ps, lhsT, rhs, start=True, stop=True)
                    dst = stage[:, yb, :, xb, :]
                    # One evac engine per y-band -> each out-DMA below
                    # waits on a single semaphore; bands alternate engines.
                    if yb % 2 == 0:
                        nc.vector.tensor_copy(dst, ps)
                    else:
                        nc.scalar.copy(dst, ps)
            # Slab gather: pixel-row q lives on partitions {q, q+16, ...}
            # (r-major pixel order), so each per-q DMA is a legal pure
            # stride-16-partition AP whose free offset q*512 rides in the
            # AP base -- and its slab [q, q+9) window rows x all xb x wx
            # is one contiguous 4608-elem run per band. Wave A (bands
            # 0..2, 2 sem waits -> SWDGE) fires after band 2 and overlaps
            # band-3 compute; wave B (band 3) is the 1.2 MB tail on the
            # two HWDGE rings.
            ROW = NTY * NY * NTX * NX                # 49152 elems/partition
            BAND = NY * NTX * NX                     # 12288
            RUN = ND * NTX * NX                      # 4608
            stage_t = stage[:, :, :, :, :].tensor
            for q in range(YB):
                src = bass_rust.AP(
                    stage_t,
                    q * ROW + q * NTX * NX,
                    [[YB * ROW, XB], [BAND, 3], [1, RUN]],
                )
                nc.gpsimd.dma_start(out_a[q], src)
            for q in range(YB):
                src = bass_rust.AP(
                    stage_t,
                    q * ROW + 3 * BAND + q * NTX * NX,
                    [[YB * ROW, XB], [1, RUN]],
                )
                eng = nc.sync if q % 2 == 0 else nc.scalar
                eng.dma_start(out_b[q], src)
    return nc


def make_in_maps(prv: np.ndarray, nxt: np.ndarray) -> list[dict[str, np.ndarray]]:
    prv = np.asarray(prv, dtype=np.float32)
    nxt = np.asarray(nxt, dtype=np.float32)
    nxt_pad = np.zeros((B, C, H + 2 * D, W + 2 * D), np.float32)
    nxt_pad[:, :, D : D + H, D : D + W] = nxt * np.float32(0.125)
    prv_s = prv * np.float32(0.0625)  # 2^-4 * 2^-3 = 1/C
    in_maps = []
    for core in range(N_CORES):
        b, h = divmod(core, 2)
        # prv tile-major, yb-outer, r-major pixels within a tile
        # (partition m = r*YB + q): [C, yb, xb, r, q]
        p = prv_s[b, :, h * HH : (h + 1) * HH, :].reshape(C, NTY, YB, NTX, XB)
        p = np.ascontiguousarray(p.transpose(0, 1, 3, 4, 2)).reshape(C, -1)
        # nxt unbanded: [C, 72, 264]
        x = nxt_pad[b, :, h * HH : h * HH + HP, :]
        in_maps.append(
            {
                "prv_s": p.astype(np.float16),
                "nxt_s": np.ascontiguousarray(x).reshape(C, -1).astype(np.float16),
            }
        )
    return in_maps


def extract_core(A: np.ndarray, Bw: np.ndarray) -> np.ndarray:
    """Two-wave slab dump -> [K, HH, W] fp32.

    A[q, r, yb(3), j, xb, wx] / Bw[q, r, j, xb, wx] hold psum col
    (q+j)*NX + wx of pixel (q, r) in tile (yb, xb); displacement
    k=(dy,dx) is at j=dy, wx = r + dx.
    """
    A = np.asarray(A).astype(np.float32).reshape(YB, XB, 3, ND, NTX, NX)
    Bw = np.asarray(Bw).astype(np.float32).reshape(YB, XB, 1, ND, NTX, NX)
    G = np.concatenate([A.transpose(2, 0, 1, 3, 4, 5),
                        Bw.transpose(2, 0, 1, 3, 4, 5)], axis=0)
    G = G.transpose(0, 1, 4, 2, 3, 5)                 # [yb, q, xb, r, j, wx]
    dy, dx = np.divmod(np.arange(K), ND)              # [81]
    r = np.arange(XB)
    ridx = np.broadcast_to(r[None, :], (K, XB))       # [81, 8]
    jidx = np.broadcast_to(dy[:, None], (K, XB))      # [81, 8]
    wxidx = r[None, :] + dx[:, None]                  # [81, 8]
    T = G[:, :, :, ridx, jidx, wxidx]                 # [yb, q, xb, 81, r]
    T = T.transpose(3, 0, 1, 2, 4)                    # [81, yb, q, xb, r]
    return T.reshape(K, HH, W)


def run(prv: np.ndarray, nxt: np.ndarray, trace: bool = False):
    nc = build_nc()
    nc.finalize()
    in_maps = make_in_maps(prv, nxt)
    res = run_bass_kernel_spmd(nc, in_maps, list(range(N_CORES)), trace=trace)
    out = np.empty((B, K, H, W), np.float32)
    for core in range(N_CORES):
        b, h = divmod(core, 2)
        out[b, :, h * HH : (h + 1) * HH, :] = extract_core(
            res.results[core]["out_a"], res.results[core]["out_b"]
        )
    return out, res


def kernel(prv: np.ndarray, nxt: np.ndarray) -> np.ndarray:
    out, _ = run(prv, nxt, trace=False)
    return out


if __name__ == "__main__":
    rng = np.random.default_rng(0)
    prv = rng.standard_normal((B, C, H, W), dtype=np.float32)
    nxt = rng.standard_normal((B, C, H, W), dtype=np.float32)
    out = kernel(prv, nxt)
    print(out.shape, out.dtype)



# revision 3
# speedup vs baseline: 1.4592x; 1.4592x over previous
"""Cost volume (tfa CorrelationCost, kernel_size=1, d=4) on 8 TRN2 cores.

out[b, k, y, x] = (1/C) * sum_c prv[b,c,y,x] * nxt_pad[b,c,y+dy,x+dx],
k = dy*9+dx, dy/dx in 0..8, nxt zero-padded by 4 on each spatial side.

Sharding: core i -> (batch b = i//2, H-half h = i%2). Each core gets the
full-C feature maps for its 64 rows (prv) and 72 padded rows (nxt).

Per-core algorithm (fp16 banded matmul), v5 -- latency/overlap-optimized
rewrite of v4 (93us). Three structural fixes over v4, driven by the trace:

1. v4 issued all 7 input DMAs up front across two queues; packet-level
   round-robin let every chunk share HBM bandwidth equally, so band-0's
   data landed only ~1us before ALL input data (first matmul at 25.4us).
   v5 puts every input DMA on the single gpsimd SWDGE queue in band
   order: in-order draining gives band 0 its 2.67MB at full line rate
   (~8us in), and compute overlaps the remaining input stream.

2. v4's evac (PSUM->SBUF fp32->fp16 cast) alternated whole bands between
   vector and scalar, one [128,384] op per tile: per-op fixed cost
   (120cyc DVE / 172cyc ACT) + sem made the steady-state tile pitch
   ~470ns and the compute phase 53us. v5 evacs PAIRS of tiles (two psum
   banks, [128,2,512] psum tiles; matmuls write cols 0:384 of each bank)
   in one op, alternating engines per pair: DVE (120+768)/0.96 ~ 925ns
   and ACT (172+768)/1.2 ~ 783ns per 2 tiles run concurrently -> ~245ns
   per tile.

3. v4's output used 8-partition stride-16 DMAs; 8 partitions map to only
   ~4 of 16 SBUF AXI ports, so the out phase ran at 120-210 GB/s and was
   an unoverlapped 22us tail. v5 reorders pixels q-major (partition
   m = 8q+r) so each band's entire slab dump is ONE 128-consecutive-
   partition DMA with a mixed partition+offset stride for the q dim
   (stride 8*ROW+512: 8 partitions down, 512 elems right -- the wy in
   [q, q+9) slab window), hitting all 16 ports at line rate. 4 output
   DMAs total, queued on the same SWDGE queue behind the inputs, so
   bands 0-2 drain during compute and only band 3 (1.18MB) is tail.

Traffic per core: prv 4.19MB + nxt 4.86MB + out 4.72MB = 13.8MB.
"""

import numpy as np

import bass_rust
import concourse.bass as bass
import concourse.tile as tile
from concourse import bacc, mybir
from concourse.bass_utils import run_bass_kernel_spmd

# Problem geometry (hardcoded per spec)
B, C, H, W = 4, 128, 128, 256
D = 4
ND = 2 * D + 1            # 9
K = ND * ND               # 81
HH = H // 2               # 64 rows per core
HP = HH + 2 * D           # 72 padded nxt rows per core
WP = W + 2 * D            # 264 padded nxt cols
YB, XB = 16, 8            # pixel tile: 16 rows x 8 cols = 128 partitions
NY, NX = YB + 2 * D, XB + 2 * D   # 24 x 16 window
NTY, NTX = HH // YB, W // XB      # 4 y-bands x 32 x-tiles
NWIN = NY * NX            # 384
N_CORES = 8

ROW = NTY * NY * NTX * NX         # 49152 stage elems per partition
BAND = NY * NTX * NX              # 12288
RUN = ND * NTX * NX               # 4608 (one slab run: 9 wy rows x 32 xb x 16 wx)

F16 = mybir.dt.float16
F32 = mybir.dt.float32

# Output DMA scheme: "mixed" = one 128-partition DMA per band using a
# mixed partition+offset stride for the q dim (exact 9-row slabs);
# "quad" = 4 DMAs per band over 32-consecutive-partition quads dumping
# 12 wy rows (host picks the 9 needed) -- fallback if mixed is rejected.
OUT_SCHEME = "quad"  # "mixed" is rejected by the BIR verifier (illegal partition step)


def build_nc():
    nc = bacc.Bacc("TRN2")
    prv_d = nc.declare_dram_parameter("prv_s", [C, NTY * NTX * 128], F16, isOutput=False)
    nxt_d = nc.declare_dram_parameter("nxt_s", [C, HP * WP], F16, isOutput=False)
    if OUT_SCHEME == "mixed":
        # out[q, r, band, run]: run = dy*512 + xb*16 + wx
        out_d = nc.declare_dram_parameter("out_s", [YB, XB, NTY, RUN], F16, isOutput=True)
    else:
        # out[band, quad, part-in-quad, 12*512]
        out_d = nc.declare_dram_parameter(
            "out_s", [NTY, 4, 32, 12 * NTX * NX], F16, isOutput=True
        )

    with tile.TileContext(nc) as tc:
        with (
            tc.tile_pool(name="inp", bufs=1) as inp,
            tc.tile_pool(name="psum", bufs=4, space="PSUM") as pp,
            tc.tile_pool(name="stage", bufs=1) as sp,
        ):
            prv_sb = inp.tile([C, NTY * NTX * 128], F16)
            nxt_sb = inp.tile([C, HP, WP], F16)
            # stage[part, yb, wy, xb, wx]: pixel (q, r) lives on partition
            # m = 8q + r; its 81 useful values are in wy rows [q, q+9).
            stage = sp.tile([128, NTY, NY, NTX, NX], F16)

            # All input DMAs on the single gpsimd SWDGE queue in band
            # order: single-queue FIFO draining means band 0's inputs get
            # the full HBM bandwidth and complete first (~8us), instead of
            # fair-sharing with every later chunk like v4.
            def nxt_chunk(j):  # 24-row chunks; band yb needs rows [16yb, 16yb+24)
                nc.gpsimd.dma_start(
                    nxt_sb[:, 24 * j : 24 * j + 24, :],
                    nxt_d[:, 24 * j * WP : (24 * j + 24) * WP],
                )

            def prv_chunk(j):  # per-band chunks of 32 tiles
                lo = j * NTX * 128
                nc.gpsimd.dma_start(
                    prv_sb[:, lo : lo + NTX * 128], prv_d[:, lo : lo + NTX * 128]
                )

            nxt_chunk(0)
            prv_chunk(0)
            nxt_chunk(1)
            prv_chunk(1)
            nxt_chunk(2)
            prv_chunk(2)
            prv_chunk(3)

            stage_t = stage[:, :, :, :, :].tensor

            for yb in range(NTY):
                # Absorb input-DMA waits on cheap PE instructions so each
                # matmul below carries only its psum-slot release wait.
                nc.tensor.ldweights(prv_sb[:, yb * NTX * 128 : yb * NTX * 128 + 1])
                nc.tensor.ldweights(nxt_sb[:, 16 * yb, :1])
                nc.tensor.ldweights(nxt_sb[:, 16 * yb + 23, :1])
                for xp in range(NTX // 2):
                    # One psum tile = 2 banks; matmul t writes bank t's
                    # cols [0, 384) -- both matmuls then evac'd in ONE op.
                    ps = pp.tile([128, 2, 512], F32)
                    for t in range(2):
                        xb = 2 * xp + t
                        ti = yb * NTX + xb
                        lhsT = prv_sb[:, ti * 128 : (ti + 1) * 128]
                        rhs = nxt_sb[:, yb * YB : yb * YB + NY, xb * XB : xb * XB + NX]
                        nc.tensor.matmul(ps[:, t, 0:NWIN], lhsT, rhs, start=True, stop=True)
                    # Evac both tiles in one strided op: src iterates
                    # (wy, t, wx) to match the stage's xb-major layout.
                    src = bass_rust.AP(
                        ps[:, :, :].tensor,
                        0,
                        [[2 * 512, 128], [NX, NY], [512, 2], [1, NX]],
                    )
                    dst = stage[:, yb, :, 2 * xp : 2 * xp + 2, :]
                    pi = yb * (NTX // 2) + xp
                    if pi % 2 == 0:
                        nc.vector.tensor_copy(dst, src)
                    else:
                        nc.scalar.copy(dst, src)

            # Slab dump, one DMA per band (128 consecutive partitions ->
            # all 16 SBUF AXI ports -> line rate). Queued on the same
            # SWDGE queue behind the inputs: bands 0-2 drain during
            # compute; band 3 is the only tail.
            if OUT_SCHEME == "mixed":
                for b in range(NTY):
                    src = bass_rust.AP(
                        stage_t,
                        b * BAND,
                        [[8 * ROW + NTX * NX, YB], [ROW, XB], [1, RUN]],
                    )
                    nc.gpsimd.dma_start(out_d[:, :, b, :], src)
            else:
                for b in range(NTY):
                    for i in range(4):
                        src = bass_rust.AP(
                            stage_t,
                            32 * i * ROW + b * BAND + 4 * i * NTX * NX,
                            [[ROW, 32], [1, 12 * NTX * NX]],
                        )
                        nc.gpsimd.dma_start(out_d[b, i], src)
    return nc


def make_in_maps(prv: np.ndarray, nxt: np.ndarray) -> list[dict[str, np.ndarray]]:
    prv = np.asarray(prv, dtype=np.float32)
    nxt = np.asarray(nxt, dtype=np.float32)
    nxt_pad = np.zeros((B, C, H + 2 * D, W + 2 * D), np.float32)
    nxt_pad[:, :, D : D + H, D : D + W] = nxt * np.float32(0.125)
    prv_s = prv * np.float32(0.0625)  # 2^-4 * 2^-3 = 1/C
    in_maps = []
    for core in range(N_CORES):
        b, h = divmod(core, 2)
        # prv tile-major, yb-outer, q-MAJOR pixels within a tile
        # (partition m = 8q + r): [C, yb, xb, q, r]
        p = prv_s[b, :, h * HH : (h + 1) * HH, :].reshape(C, NTY, YB, NTX, XB)
        p = np.ascontiguousarray(p.transpose(0, 1, 3, 2, 4)).reshape(C, -1)
        # nxt unbanded: [C, 72, 264]
        x = nxt_pad[b, :, h * HH : h * HH + HP, :]
        in_maps.append(
            {
                "prv_s": p.astype(np.float16),
                "nxt_s": np.ascontiguousarray(x).reshape(C, -1).astype(np.float16),
            }
        )
    return in_maps


def extract_core(O: np.ndarray) -> np.ndarray:
    """Slab dump -> [K, HH, W] fp32.

    mixed: O[q, r, band, dy*512 + xb*16 + wx] holds psum col
    (q+dy)*16 + wx of pixel (y=16*band+q, x=8*xb+r); displacement
    k=(dy,dx) is at wx = r + dx.
    quad:  O[band, quad, p, j*512 + xb*16 + wx] with p = partition-32*quad
    = 8*(q-4*quad)+r ... j = wy - 4*quad, so dy = j - (q - 4*quad).
    """
    dy, dx = np.divmod(np.arange(K), ND)              # [81]
    r = np.arange(XB)
    if OUT_SCHEME == "mixed":
        A = np.asarray(O).astype(np.float32).reshape(YB, XB, NTY, ND, NTX, NX)
        G = A.transpose(2, 0, 4, 1, 3, 5)             # [band, q, xb, r, dy, wx]
        ridx = np.broadcast_to(r[None, :], (K, XB))   # [81, 8]
        jidx = np.broadcast_to(dy[:, None], (K, XB))  # [81, 8]
        wxidx = r[None, :] + dx[:, None]              # [81, 8]
        T = G[:, :, :, ridx, jidx, wxidx]             # [band, q, xb, 81, r]
        T = T.transpose(3, 0, 1, 2, 4)                # [81, band, q, xb, r]
        return T.reshape(K, HH, W)
    else:
        A = np.asarray(O).astype(np.float32).reshape(NTY, 4, 4, XB, 12, NTX, NX)
        # A[band, quad, qq, r, j, xb, wx]: pixel q = 4*quad + qq,
        # wy = 4*quad + j -> dy = j - qq
        G = A.transpose(0, 1, 2, 5, 3, 4, 6)          # [band, quad, qq, xb, r, j, wx]
        ridx = np.broadcast_to(r[None, :], (K, XB))
        wxidx = r[None, :] + dx[:, None]
        out = np.empty((NTY, 4, 4, NTX, K, XB), np.float32)
        for qq in range(4):
            jidx = np.broadcast_to(dy[:, None] + qq, (K, XB))
            out[:, :, qq] = G[:, :, qq][:, :, :, ridx, jidx, wxidx]
        T = out.transpose(4, 0, 1, 2, 3, 5)           # [81, band, quad, qq, xb, r]
        return T.reshape(K, HH, W)


def run(prv: np.ndarray, nxt: np.ndarray, trace: bool = False):
    nc = build_nc()
    nc.finalize()
    in_maps = make_in_maps(prv, nxt)
    res = run_bass_kernel_spmd(nc, in_maps, list(range(N_CORES)), trace=trace)
    out = np.empty((B, K, H, W), np.float32)
    for core in range(N_CORES):
        b, h = divmod(core, 2)
        out[b, :, h * HH : (h + 1) * HH, :] = extract_core(res.results[core]["out_s"])
    return out, res


def kernel(prv: np.ndarray, nxt: np.ndarray) -> np.ndarray:
    out, _ = run(prv, nxt, trace=False)
    return out


if __name__ == "__main__":
    rng = np.random.default_rng(0)
    prv = rng.standard_normal((B, C, H, W), dtype=np.float32)
    nxt = rng.standard_normal((B, C, H, W), dtype=np.float32)
    out = kernel(prv, nxt)
    print(out.shape, out.dtype)
